# revision 51
# baseline (speedup 1.0000x reference)
"""Trainium2 Bass kernel for nn_ConvAttention (dwconv3x3->BN->GELU->1x1 conv
q/k/v branches, 8-head attention over 32x32 tokens, 1x1 out-proj, BN).

Sharding: data-parallel over batch B=8 across the 8 NeuronCores (one image
per core). Training-mode BatchNorm stats are computed exactly on the host
(numpy recompute of the depthwise conv on bf16 inputs, matching the device);
the final BN is applied on the host after gathering.

Device per-core pipeline (heavy matmuls bf16; Tile list-scheduler orders
ready work by emission index, so emission order encodes priority =
criticality):
  lead-in: x arrives pre-padded bf16 [128,34,34]/block; 9 diagonal-matmul
  depthwise convs (fused BN+GELU on ACT) + q/k pointwise, PE-continuous.
  attention: per-pair "windows" of 16 S->exp units (S^T chunk = k^T q,
  2 matmuls K=48 at PE row-tiles 0/64; exp on ACT [128,1024] paces the
  whole phase at ~1.08us/unit). Lower-priority fillers pack the PE slack
  inside each window: v^T production (v pointwise TRANSPOSED into
  [128, pair, 113] tiles: [den0@0|h0@1-48|zeros|den1@64|h1@65-112], den
  cols memset 1.0), next pair's q/k pointwise, and the PREVIOUS pair's O
  matmuls (M=49 per (j,hh,nch) accumulating each head's O and softmax
  denominator at pO rows 0/64). Window 3 interleaves its own O's; O(2)
  drains at the 2->3 boundary.
  divide: O[:, n] /= d[n] via exp(-ln d) (ln/exp share an ACT table): one
  ln op covers both denominator rows (ACT cost is free-size based); the
  broadcast to 64-row bands goes through a DRAM bounce + 0-stride DMA
  re-read (pairs 0-2, latency hidden) or a (-1s)-matmul (pair 3, tail);
  ACT exp, DVE multiplies. The chain stays fp32: O has a large common-mode
  component and the final BN divides by a small per-channel variance,
  amplifying coherent error ~5x.
  out-proj (f32r): m=0 pre-accumulates over pairs 0-2 as window-3 filler,
  m=1/2 pre-accumulate in freed ps-pool PSUM slots overlapping divide(3);
  pair-3 chunks close after the last divide; split copies/DMAs drain out.
  PSUM: 2 general rotating slots (S tiles) + 2 O/pointwise/out-proj slots.
  A post-schedule pass dedupes ldweights for consecutive same-stationary
  matmuls and splits multi-wait instructions for old walrus.
"""

import sys

sys.path.insert(0, "/opt/trn_rl_repo")

import numpy as np
import ml_dtypes

import concourse.bass as bass
import concourse.mybir as mybir
import concourse.tile as tile
from concourse.bass_utils import run_bass_kernel_spmd

BF16 = ml_dtypes.bfloat16
F32 = mybir.dt.float32
BF = mybir.dt.bfloat16
F32R = mybir.dt.float32r

B, C, H, W = 8, 384, 32, 32
N = H * W
HEADS, HD = 8, 48
SCALE = float(HD ** -0.5)
NBLK = C // 128          # 3 channel blocks
NPAIR = HEADS // 2       # 4 head pairs
VTW = 113                # vt cols per pair-half set: den0,48,zeros15,den1,48
EPS = 1e-5

_GELU = mybir.ActivationFunctionType.Gelu
_EXP = mybir.ActivationFunctionType.Exp
_LN = mybir.ActivationFunctionType.Ln


# ---------------------------------------------------------------- wait split
def _split_excess_waits(nc, max_waits=1):
    """Old walrus rejects >1 sync wait per instruction; hoist extras onto
    NoOps inserted just before, on the same engine (queue order preserved)."""
    n = 0
    for f in nc.m.functions:
        for bb in f.blocks:
            out, changed = [], False
            for inst in bb.instructions:
                si = inst.sync_info
                waits = list(si.on_wait) if si is not None else []
                if len(waits) > max_waits:
                    excess, keep = waits[:-max_waits], waits[-max_waits:]
                    for j, w in enumerate(excess):
                        nop = mybir.InstNoOp(
                            name=f"WSPLIT-{inst.name}-{j}", ins=[], outs=[])
                        nop.engine = inst.engine
                        nop.sync_info = mybir.SyncInfo(on_wait=[w], on_update=[])
                        out.append(nop)
                        n += 1
                    inst.sync_info = mybir.SyncInfo(
                        on_wait=keep, on_update=list(si.on_update))
                    changed = True
                out.append(inst)
            if changed:
                bb.instructions = out
    return n



# ---------------------------------------------------------- ldweights dedupe
def _dedupe_ldweights(nc):
    """After scheduling, consecutive PE matmuls that use the identical
    stationary operand don't need to reload it: set ldweights=False on the
    followers. Any other PE instruction between them is a barrier."""
    def sig(inst):
        w = inst.ins[1]
        try:
            return (str(w), inst.perf_mode, inst.is_transpose,
                    tuple(inst.tile_position or ()))
        except Exception:
            return None
    n = 0
    for f in nc.m.functions:
        for bb in f.blocks:
            last = {}
            for inst in bb.instructions:
                eng = inst.engine
                if not isinstance(inst, mybir.InstMatmult):
                    if eng in last and not isinstance(inst, mybir.InstNoOp):
                        last.pop(eng, None)
                    continue
                s = sig(inst)
                if s is not None and last.get(eng) == s:
                    inst.ldweights = False
                    n += 1
                else:
                    last[eng] = s
    return n


# ---------------------------------------------------------------- builder
def build_kernel(split_waits=True, debug_tap=None):
    nc = bass.Bass("TRN2", target_bir_lowering=False, debug=False)
    dbg_d = None
    if debug_tap == "osb":
        dbg_d = nc.dram_tensor("dbg", [NPAIR, 128, N], F32,
                               kind="ExternalOutput").ap()
    elif debug_tap == "qk":
        dbg_d = nc.dram_tensor("dbg", [2, NPAIR, 112, N], BF,
                               kind="ExternalOutput").ap()
    elif debug_tap == "vt":
        dbg_d = nc.dram_tensor("dbg", [8, 128, NPAIR * VTW], BF,
                               kind="ExternalOutput").ap()
    elif debug_tap == "pt0":
        dbg_d = nc.dram_tensor("dbg", [16, 128, N], BF,
                               kind="ExternalOutput").ap()
    elif debug_tap == "po":
        dbg_d = nc.dram_tensor("dbg", [NPAIR, 128, N], F32,
                               kind="ExternalOutput").ap()

    xpad_d = nc.dram_tensor("xpad", [NBLK, 128, H + 2, W + 2], BF,
                            kind="ExternalInput").ap()
    diag_d = nc.dram_tensor("diags", [3, NBLK, 128, 9, 128], BF,
                            kind="ExternalInput").ap()
    # A/D packed: [128, br*blk*2] (col = 2*(3*br+blk) + {0:A,1:D})
    AD_d = nc.dram_tensor("scaleAD", [128, 18], F32,
                          kind="ExternalInput").ap()
    # q,k pointwise: [2, kc, 128, pair, 112] bf16 (head0 cols 0-47, head1 64-111)
    pwqkT_d = nc.dram_tensor("pwqkT", [2, NBLK, 128, NPAIR, 112], BF,
                             kind="ExternalInput").ap()
    # v pointwise transposed-producing: [kc, 128, pair, VTW] bf16
    # cols 1-48 = head0 dims, 65-112 = head1 dims, rest zero
    pwvT_d = nc.dram_tensor("pwvT", [NBLK, 128, NPAIR, VTW], BF,
                            kind="ExternalInput").ap()
    # out-proj per pair: [pair, 128, C]; rows 0,49-64,113-127 ZERO,
    # rows 1-48 / 65-112 = the two heads' weights (denom rows 0/64 in osb)
    woPairT_d = nc.dram_tensor("woPairT", [NPAIR, 128, C], F32R,
                               kind="ExternalInput").ap()
    ones_d = nc.dram_tensor("onesc", [65, 64], F32,
                            kind="ExternalInput").ap()
    out_d = nc.dram_tensor("out", [C, N], F32, kind="ExternalOutput").ap()
    bounce_d = nc.dram_tensor("lnbounce", [NPAIR, 2, N], F32,
                              kind="Internal").ap()

    with tile.TileContext(nc) as tc:
        from contextlib import ExitStack
        ctx = ExitStack()
        with ctx:
            cpool = ctx.enter_context(tc.tile_pool(name="consts", bufs=1))
            yhpool = ctx.enter_context(tc.tile_pool(name="yh", bufs=1))
            qkpool = ctx.enter_context(tc.tile_pool(name="qk", bufs=1))
            vtpool = ctx.enter_context(tc.tile_pool(name="vt", bufs=1))
            ptpool = ctx.enter_context(tc.tile_pool(name="pt", bufs=36))
            osbpool = ctx.enter_context(tc.tile_pool(name="osb", bufs=1))
            rbpool = ctx.enter_context(tc.tile_pool(name="rb", bufs=2))
            outpool = ctx.enter_context(tc.tile_pool(name="outsb", bufs=3))

            # PSUM: 2 rotating general slots (4 banks) + 2 rotating O/outproj
            # slots (4 banks) = all 8 banks
            pspool = ctx.enter_context(
                tc.tile_pool(name="ps", bufs=2, space="PSUM"))
            popool = ctx.enter_context(
                tc.tile_pool(name="po", bufs=2, space="PSUM"))

            # ---------------- constants (first conv's deps DMA'd first)
            xpad = {}
            diag_t = {}
            for blk in range(NBLK):
                t = cpool.tile([128, H + 2, W + 2], BF, tag=f"xpad{blk}",
                               name=f"xpad{blk}")
                d0 = cpool.tile([128, 9, 128], BF, tag=f"diag0_{blk}",
                                name=f"diag0_{blk}")
                if blk == 0:
                    # first conv's deps: slab-split for queue parallelism
                    for p8 in range(8):
                        sl = slice(16 * p8, 16 * p8 + 16)
                        nc.sync.dma_start(t[sl, :, :], xpad_d[blk, sl])
                        nc.sync.dma_start(d0[sl, :, :], diag_d[0, blk, sl])
                else:
                    nc.sync.dma_start(t[:], xpad_d[blk])
                    nc.sync.dma_start(d0[:], diag_d[0, blk])
                xpad[blk] = t
                diag_t[(0, blk)] = d0
            AD_t = cpool.tile([128, 18], F32, tag="AD")
            nc.sync.dma_start(AD_t[:], AD_d[:])
            for br in (1, 2):
                for blk in range(NBLK):
                    t = cpool.tile([128, 9, 128], BF, tag=f"diag{br}_{blk}",
                                   name=f"diag{br}_{blk}")
                    nc.sync.dma_start(t[:], diag_d[br, blk])
                    diag_t[(br, blk)] = t
            ones_f = cpool.tile([65, 64], F32, tag="ones")
            nc.sync.dma_start(ones_f[:], ones_d[:])
            pwqk_t = {}
            for br in range(2):
                for kc in range(NBLK):
                    t = cpool.tile([128, NPAIR, 112], BF,
                                   tag=f"pwqk{br}_{kc}", name=f"pwqk{br}_{kc}")
                    nc.sync.dma_start(t[:], pwqkT_d[br, kc])
                    pwqk_t[(br, kc)] = t
            pwv_t = {}
            for kc in range(NBLK):
                t = cpool.tile([128, NPAIR, VTW], BF, tag=f"pwv{kc}",
                               name=f"pwv{kc}")
                nc.sync.dma_start(t[:], pwvT_d[kc])
                pwv_t[kc] = t
            wo_t = {}
            for pair in range(NPAIR):
                t = cpool.tile([128, C], F32R, tag=f"wo{pair}",
                               name=f"wo{pair}")
                nc.sync.dma_start(t[:], woPairT_d[pair])
                wo_t[pair] = t

            # osb pair tiles zeroed once on gpsimd: only rows 1-48/65-112 are
            # written by the divide; out-proj contracts all 128 rows and
            # 0*garbage(NaN) would poison it.
            osb_t = {}
            for pair in range(NPAIR):
                t = osbpool.tile([128, N], F32R, tag=f"osb{pair}",
                                 name=f"osb{pair}")
                nc.vector.memset(t[:].bitcast(F32), 0.0)
                osb_t[pair] = t

            # ---------------- depthwise conv + BN + GELU
            yh_t = {}

            def conv_bb(br, blk):
                py = pspool.tile([128, N], F32, tag="ps", name=f"py{br}_{blk}")
                for tap in range(9):
                    di, dj = tap // 3, tap % 3
                    for hf in range(2):
                        nc.tensor.matmul(
                            py[:, hf * 512:(hf + 1) * 512],
                            diag_t[(br, blk)][:, tap, :],
                            xpad[blk][:, di + 16 * hf:di + 16 * hf + 16,
                                      dj:dj + W],
                            start=(tap == 0), stop=(tap == 8))
                yh = yhpool.tile([128, N], BF, tag=f"yh{br}_{blk}",
                                 name=f"yh{br}_{blk}")
                col = 2 * (3 * br + blk)
                nc.scalar.activation(
                    yh[:], py[:], _GELU,
                    bias=AD_t[:, col + 1:col + 2],
                    scale=AD_t[:, col:col + 1])
                yh_t[(br, blk)] = yh

            # ---------------- pointwise q,k (bf16, M=112 head-pairs)
            qk_sb = {}

            def pw_branch(br, pair):
                pp = popool.tile([128, N], F32, tag="po",
                                 name=f"pp{br}_{pair}")
                for kc in range(NBLK):
                    lhsT = pwqk_t[(br, kc)][:, pair, :]
                    for nch in range(2):
                        nc.tensor.matmul(
                            pp[0:112, nch * 512:(nch + 1) * 512],
                            lhsT,
                            yh_t[(br, kc)][:, nch * 512:(nch + 1) * 512],
                            start=(kc == 0), stop=(kc == NBLK - 1))
                sb = qkpool.tile([112, N], BF, tag=f"qk{br}_{pair}",
                                 name=f"qk{br}_{pair}")
                nc.vector.tensor_copy(sb[:], pp[0:112, :])
                qk_sb[(br, pair)] = sb

            # ---------------- v^T tiles (one per key chunk j)
            vt_t = {}

            def make_vt(j):
                pv = popool.tile([128, N], F32, tag="po",
                                 name=f"pv{j}")
                for kc in range(NBLK):
                    nc.tensor.matmul(
                        pv[:, 0:NPAIR * VTW],
                        yh_t[(2, kc)][:, j * 128:(j + 1) * 128],
                        pwv_t[kc][:, :, :],
                        start=(kc == 0), stop=(kc == NBLK - 1))
                vt = vtpool.tile([128, NPAIR, VTW], BF, tag=f"vt{j}",
                                 name=f"vt{j}")
                nc.vector.tensor_copy(vt[:], pv[:, 0:NPAIR * VTW])
                nc.gpsimd.memset(vt[:, :, 0:1], 1.0)
                nc.gpsimd.memset(vt[:, :, 64:65], 1.0)
                vt_t[j] = vt

            # ---------------- attention units
            pts_all = {}

            def S_unit(pair, u):
                j, hh = u // 2, u % 2
                off = 64 * hh
                q_sb = qk_sb[(0, pair)]
                k_sb = qk_sb[(1, pair)]
                pS = pspool.tile([128, N], F32, tag="ps",
                                 name=f"pS{pair}_{u}")
                for nch in range(2):
                    nc.tensor.matmul(
                        pS[:, nch * 512:(nch + 1) * 512],
                        k_sb[off:off + 48, j * 128:(j + 1) * 128],
                        q_sb[off:off + 48, nch * 512:(nch + 1) * 512],
                        start=True, stop=True)
                pt = ptpool.tile([128, N], BF, tag="pt",
                                 name=f"pt{pair}_{u}")
                nc.scalar.activation(pt[:], pS[:], _EXP, bias=0.0,
                                     scale=SCALE)
                pts_all[(pair, u)] = pt

            pO_t = {}

            def O_unit2(pair, j):
                # per (j): each head's pt needs its own matmul (P differs
                # per head!) — M=49 [den_h | dims_h] at out rows 64*hh.
                if j == 0:
                    pO_t[pair] = popool.tile([128, N], F32, tag="po",
                                             name=f"pO{pair}")
                    # rows 49-63 are read by the [0:64] divide band but
                    # never written by the M=49 matmuls; zero them once
                    # (rows 32-48 get overwritten by the matmuls).
                    nc.vector.memset(pO_t[pair][32:64, :], 0.0)
                pO = pO_t[pair]
                for hh in range(2):
                    pt = pts_all[(pair, 2 * j + hh)]
                    lhsT = vt_t[j][:, pair, 64 * hh:64 * hh + 49]
                    for nch in range(2):
                        nc.tensor.matmul(
                            pO[64 * hh:64 * hh + 49,
                               nch * 512:(nch + 1) * 512],
                            lhsT,
                            pt[:, nch * 512:(nch + 1) * 512],
                            start=(j == 0), stop=(j == 7))

            rb_t = {}

            def divide_ln(pair):
                """O[:, n] /= d[n] via exp(-ln d); ln/exp share an ACT table
                so no swaps mid-stream. This half: ln of the two denominator
                rows (pO rows {0,64}) -> rb; runs as soon as pO stops."""
                pO = pO_t[pair]
                rb = rbpool.tile([65, N], F32, tag="rb", name=f"rb{pair}")
                # one op over rows 0-64: ACT cost is free-size based, so
                # this prices the same as a single row; rows 1-63 become
                # ln(O) garbage (NaN for negatives) but are never read.
                nc.scalar.activation(rb[0:65, :], pO[0:65, :], _LN,
                                     bias=0.0, scale=1.0)
                rb_t[pair] = rb

            def divide_bc(pair):
                """Broadcast ln d to 64-row bands without touching the PE:
                bounce the two rb rows through DRAM, re-read each with a
                0-stride DMA, then ACT exp(-lnd) and DVE multiplies into
                osb. Chain stays fp32 (O has a large common-mode component;
                the final BN amplifies coherent error ~5x)."""
                pO = pO_t[pair]
                rb = rb_t[pair]
                nc.sync.dma_start(bounce_d[pair, 0], rb[0:1, :])
                nc.sync.dma_start(bounce_d[pair, 1], rb[64:65, :])
                bcl = rbpool.tile([128, N], F32, tag="bcl",
                                  name=f"bcl{pair}")
                nc.sync.dma_start(
                    bcl[0:64, :],
                    bounce_d[pair, 0:1, :].to_broadcast([64, N]))
                nc.sync.dma_start(
                    bcl[64:128, :],
                    bounce_d[pair, 1:2, :].to_broadcast([64, N]))
                bc = rbpool.tile([128, N], F32, tag="bc", name=f"bc{pair}")
                nc.scalar.activation(bc[:], bcl[:], _EXP, bias=0.0,
                                     scale=-1.0)
                # PSUM partition starts must be 32-aligned; rows 0/64 hold
                # the denominators (d*r = ~1, out-proj weight rows are 0)
                # and rows 49-63 were zeroed at pO alloc. Rows 113-127 of
                # pO are uninitialized -- never read.
                nc.vector.tensor_mul(
                    osb_t[pair][0:64, :], pO[0:64, :], bc[0:64, :])
                nc.vector.tensor_mul(
                    osb_t[pair][64:96, :], pO[64:96, :], bc[64:96, :])
                nc.vector.tensor_mul(
                    osb_t[pair][96:113, :], pO[96:113, :], bc[96:113, :])

            def divide_pb(pair):
                """Tail path: (-1s)-matmul broadcast (fp32, PE idle by
                then; lower latency than the DRAM bounce), ACT exp, DVE
                multiplies."""
                pO = pO_t[pair]
                rb = rb_t[pair]
                pb = pspool.tile([128, N], F32, tag="ps", name=f"pb{pair}")
                for nch in range(2):
                    nc.tensor.matmul(
                        pb[0:64, nch * 512:(nch + 1) * 512],
                        ones_f[0:1, :],
                        rb[0:1, nch * 512:(nch + 1) * 512],
                        start=True, stop=True)
                    nc.tensor.matmul(
                        pb[64:128, nch * 512:(nch + 1) * 512],
                        ones_f[64:65, :],
                        rb[64:65, nch * 512:(nch + 1) * 512],
                        start=True, stop=True)
                bc = rbpool.tile([128, N], F32, tag="bc", name=f"bc{pair}")
                nc.scalar.activation(bc[:], pb[:], _EXP, bias=0.0, scale=1.0)
                nc.vector.tensor_mul(
                    osb_t[pair][0:64, :], pO[0:64, :], bc[0:64, :])
                nc.vector.tensor_mul(
                    osb_t[pair][64:96, :], pO[64:96, :], bc[64:96, :])
                nc.vector.tensor_mul(
                    osb_t[pair][96:113, :], pO[96:113, :], bc[96:113, :])

            # ---------------- emission schedule -------------------------
            # The Tile list-scheduler orders ready instructions by emission
            # index (priority). Emit the latency-critical S->exp stream of
            # each pair BEFORE the previous pair's O/divide work so ACT is
            # always fed; O/pw/divide fill PE slack at lower priority.
            for br in range(2):
                for blk in range(NBLK):
                    conv_bb(br, blk)
            pw_branch(0, 0)
            pw_branch(1, 0)
            for blk in range(NBLK):
                conv_bb(2, blk)

            for u in range(16):
                S_unit(0, u)
            # fillers below window-0 priority: they run in its PE slack
            for j in range(8):
                make_vt(j)
            pw_branch(0, 1)
            pw_branch(1, 1)

            # windows 1,2 absorb O(0),O(1) as slack fillers; window 3
            # runs O(3) fine-interleaved (so only divide(3) trails) with
            # O(2) draining at the 2->3 boundary / window-3 gaps.
            for pair in (0, 1):
                start_u = 2 if pair > 0 else 0
                for u in range(start_u, 16):
                    S_unit(pair + 1, u)
                if pair + 2 < NPAIR:
                    pw_branch(0, pair + 2)
                    pw_branch(1, pair + 2)
                for j in range(8):
                    O_unit2(pair, j)
                # pre-seed window(pair+2) so its exps resume while the
                # divide chain below trails at the boundary
                S_unit(pair + 2, 0)
                S_unit(pair + 2, 1)
                divide_ln(pair)
                divide_bc(pair)
            # O(2) drains as a boundary burst before window 3
            for j in range(8):
                O_unit2(2, j)
            # window 3 with O(3) interleaved at matched priority
            O_unit2(3, 0)
            for u in range(2, 16):
                S_unit(3, u)
                if u % 2 == 1 and u >= 3:
                    O_unit2(3, (u - 1) // 2)
            # divide(2): DMA-bounce, latency hidden inside window 3
            divide_ln(2)
            divide_bc(2)
            divide_ln(3)
            divide_pb(3)
            # out-proj m=0 pre-accumulated over pairs 0-2: PE filler that
            # overlaps divide(3)'s ACT/DVE chain in the tail
            po0 = popool.tile([128, N], F32, tag="po", name="poM0")
            for q in range(3):
                lhsT = wo_t[q][:, 0:128]
                for nch in range(2):
                    nc.tensor.matmul(
                        po0[:, nch * 512:(nch + 1) * 512],
                        lhsT,
                        osb_t[q][:, nch * 512:(nch + 1) * 512],
                        start=(q == 0), stop=False)

            # ---------------- out projection tail (f32r)
            # m1/m2 pre-accumulate pairs 0-2 in the freed ps-pool slots,
            # overlapping divide(3); pair-3 chunks close after mult(3).
            pre = {}
            for m in (1, 2):
                po = pspool.tile([128, N], F32, tag="ps", name=f"poM{m}")
                for q in range(3):
                    lhsT = wo_t[q][:, m * 128:(m + 1) * 128]
                    for nch in range(2):
                        nc.tensor.matmul(
                            po[:, nch * 512:(nch + 1) * 512],
                            lhsT,
                            osb_t[q][:, nch * 512:(nch + 1) * 512],
                            start=(q == 0), stop=False)
                pre[m] = po
            for nch in range(2):
                nc.tensor.matmul(
                    po0[:, nch * 512:(nch + 1) * 512],
                    wo_t[3][:, 0:128],
                    osb_t[3][:, nch * 512:(nch + 1) * 512],
                    start=False, stop=True)
            ob0 = outpool.tile([128, N], F32, tag="ob", name="ob0")
            for h in range(2):
                sl = slice(512 * h, 512 * h + 512)
                nc.vector.tensor_copy(ob0[:, sl], po0[:, sl])
                for qt in range(2):
                    q = slice(512 * h + 256 * qt, 512 * h + 256 * qt + 256)
                    nc.sync.dma_start(out_d[0:128, q], ob0[:, q])
            for m in (1, 2):
                po = pre[m]
                for nch in range(2):
                    nc.tensor.matmul(
                        po[:, nch * 512:(nch + 1) * 512],
                        wo_t[3][:, m * 128:(m + 1) * 128],
                        osb_t[3][:, nch * 512:(nch + 1) * 512],
                        start=False, stop=True)
                ob = outpool.tile([128, N], F32, tag="ob", name=f"ob{m}")
                eng = nc.scalar if m == 1 else nc.vector
                for h in range(2):
                    sl = slice(512 * h, 512 * h + 512)
                    if m == 1:
                        nc.scalar.copy(ob[:, sl], po[:, sl])
                    else:
                        nc.vector.tensor_copy(ob[:, sl], po[:, sl])
                    for qt in range(2):
                        q = slice(512 * h + 256 * qt,
                                  512 * h + 256 * qt + 256)
                        nc.sync.dma_start(
                            out_d[m * 128:(m + 1) * 128, q], ob[:, q])

    if split_waits:
        _split_excess_waits(nc)
    _dedupe_ldweights(nc)
    return nc


_NC_CACHE = {}


def _get_nc():
    if "nc" not in _NC_CACHE:
        _NC_CACHE["nc"] = build_kernel()
    return _NC_CACHE["nc"]


# ---------------------------------------------------------------- host prep
def _conv_dw_np(x, dw):
    Bx, Cx, Hx, Wx = x.shape
    xp = np.zeros((Bx, Cx, Hx + 2, Wx + 2), np.float32)
    xp[:, :, 1:Hx + 1, 1:Wx + 1] = x
    y = np.zeros((Bx, Cx, Hx, Wx), np.float32)
    for i in range(3):
        for j in range(3):
            y += dw[None, :, i, j, None, None] * \
                xp[:, :, i:i + Hx, j:j + Wx]
    return y


def _host_prep(inputs):
    x = np.asarray(inputs["x"], np.float32)
    # device consumes bf16(x); use the same for the BN statistics
    xb = x.astype(BF16).astype(np.float32)
    xpad = np.zeros((B, NBLK, 128, H + 2, W + 2), BF16)
    for blk in range(NBLK):
        xpad[:, blk, :, 1:H + 1, 1:W + 1] = \
            xb[:, blk * 128:(blk + 1) * 128].astype(BF16)
    diags = np.zeros((3, NBLK, 128, 9, 128), BF16)
    AD = np.zeros((128, 18), np.float32)
    pwqkT = np.zeros((2, NBLK, 128, NPAIR, 112), BF16)
    pwvT = np.zeros((NBLK, 128, NPAIR, VTW), BF16)
    woPairT = np.zeros((NPAIR, 128, C), np.float32)
    idx = np.arange(128)
    for br, p in enumerate(["q", "k", "v"]):
        dw = np.asarray(inputs[f"dw_{p}"], np.float32).reshape(C, 3, 3)
        dwb = dw.astype(BF16).astype(np.float32)
        y = _conv_dw_np(xb, dwb)         # matches device conv (bf16 x, w)
        m = y.astype(np.float64).mean(axis=(0, 2, 3))
        v = y.astype(np.float64).var(axis=(0, 2, 3))
        g = np.asarray(inputs[f"g_{p}"], np.float64)
        bb = np.asarray(inputs[f"b_{p}"], np.float64)
        a = g / np.sqrt(v + EPS)
        dd = (bb - m * a)
        for blk in range(NBLK):
            col = 2 * (3 * br + blk)
            AD[:, col] = a[blk * 128:(blk + 1) * 128].astype(np.float32)
            AD[:, col + 1] = dd[blk * 128:(blk + 1) * 128].astype(np.float32)
            for tap in range(9):
                diags[br, blk, idx, tap, idx] = \
                    dwb[blk * 128:(blk + 1) * 128, tap // 3, tap % 3]
        pwt = np.asarray(inputs[f"pw_{p}"], np.float32).T  # (c_in, c_out)
        for kc in range(NBLK):
            pin = pwt[kc * 128:(kc + 1) * 128]  # (128, C_out)
            for pair in range(NPAIR):
                h0 = pin[:, (2 * pair) * 48:(2 * pair + 1) * 48]
                h1 = pin[:, (2 * pair + 1) * 48:(2 * pair + 2) * 48]
                if br < 2:
                    pwqkT[br, kc, :, pair, 0:48] = h0
                    pwqkT[br, kc, :, pair, 64:112] = h1
                else:
                    pwvT[kc, :, pair, 1:49] = h0
                    pwvT[kc, :, pair, 65:113] = h1
    w_out = np.asarray(inputs["w_out"], np.float32)  # (C_out, C_in)
    for pair in range(NPAIR):
        woPairT[pair, 1:49, :] = w_out[:, (2 * pair) * 48:
                                       (2 * pair + 1) * 48].T
        woPairT[pair, 65:113, :] = w_out[:, (2 * pair + 1) * 48:
                                         (2 * pair + 2) * 48].T
    return xpad, diags, AD, pwqkT, pwvT, woPairT


def _make_in_maps(inputs):
    xpad, diags, AD, pwqkT, pwvT, woPairT = _host_prep(inputs)
    ones = np.zeros((65, 64), np.float32)
    ones[0, :] = -1.0
    ones[64, :] = -1.0
    in_maps = []
    for b in range(B):
        in_maps.append({
            "xpad": np.ascontiguousarray(xpad[b]),
            "diags": diags,
            "scaleAD": AD,
            "pwqkT": pwqkT,
            "pwvT": pwvT,
            "woPairT": woPairT,
            "onesc": ones,
        })
    return in_maps


def kernel(**inputs) -> np.ndarray:
    in_maps = _make_in_maps(inputs)
    nc = _get_nc()
    res = run_bass_kernel_spmd(nc, in_maps, list(range(B)))
    out = np.stack([res.results[b]["out"] for b in range(B)])  # (B, C, N)

    o64 = out.astype(np.float64)
    m = o64.mean(axis=(0, 2))
    v = o64.var(axis=(0, 2))
    g = np.asarray(inputs["g_out"], np.float64)
    bb = np.asarray(inputs["b_out"], np.float64)
    res_f = (o64 - m[None, :, None]) / np.sqrt(v + EPS)[None, :, None] * \
        g[None, :, None] + bb[None, :, None]
    return res_f.reshape(B, C, H, W).astype(np.float32)
